# revision 39
# baseline (speedup 1.0000x reference)
"""DIEN (GRU + AUGRU + MLP) Trainium2 Bass kernel, data-parallel over batch on 8 NeuronCores.

Hardcoded problem shape: B=4096, T=200, E=H=128, V=1e6.
Layout on device: [feature(=partition), batch] everywhere; fp16 SBUF compute dtype.

v4 structure (vs v2 baseline):
  - r/z gate pre-activations computed as ONE fp8 DoubleRow matmul per gate:
    lhsT = [Wih_g^T ; Whh_g^T] stacked on the k-tile axis, rhs = [16x ; 16h]
    fp8 halves of an XH tile -> 256-deep contraction at 2 elem/cycle.
    sigmoid descales by 1/16. n-gate (i_n, h_n) stays fp16 (precision).
  - in-place n-gate accumulation: h_n matmul opens the PSUM group, DVE
    multiplies r*h_n in place, i_n matmul accumulates on top (no separate
    npre add, no zero-matmul).
  - one batched indirect gather per step ([P,4] offsets) instead of 4.
  - scores/g/attention-gate matmuls in fp8 off the XH h-half.
  - x stored to DRAM in both fp16 (n-gate) and fp8*16 (r/z stack) for pass B.
"""

import os
import numpy as np

B, T, E, H, V = 4096, 200, 128, 128, 1000000
NCORES = 8
BC = B // NCORES           # 512 batch per core
P = 128
NCHUNK = 2
CW = BC // NCHUNK          # 256


def _build(nc, Tsteps):
    import concourse.bass as bass
    import concourse.mybir as mybir
    import concourse.tile as tile

    dt = mybir.dt
    f32 = dt.float32
    cdt = dt.float16
    f8 = dt.float8e4
    AF = mybir.ActivationFunctionType
    DR = mybir.MatmulPerfMode.DoubleRow
    ALU = mybir.AluOpType

    # ---------------- DRAM I/O ----------------
    emb16_d = nc.dram_tensor("emb16", [V, E], cdt, kind="ExternalInput")
    uh_d = nc.dram_tensor("user_hist", [BC, Tsteps], dt.int32, kind="ExternalInput")
    ad_d = nc.dram_tensor("ad_feature", [BC, 1], dt.int32, kind="ExternalInput")
    # stacked DoubleRow lhsT weights: [E, 2, H] fp8 (slot0: Wih^T, slot1: Whh^T)
    wst1r_d = nc.dram_tensor("wst1r", [E, 2, H], f8, kind="ExternalInput")
    wst1z_d = nc.dram_tensor("wst1z", [E, 2, H], f8, kind="ExternalInput")
    wst2r_d = nc.dram_tensor("wst2r", [E, 2, H], f8, kind="ExternalInput")
    wst2z_d = nc.dram_tensor("wst2z", [E, 2, H], f8, kind="ExternalInput")
    win1_d = nc.dram_tensor("win1", [E, H], cdt, kind="ExternalInput")
    whn1_d = nc.dram_tensor("whn1", [H, H], cdt, kind="ExternalInput")
    win2_d = nc.dram_tensor("win2", [E, H], cdt, kind="ExternalInput")
    whn2_d = nc.dram_tensor("whn2", [H, H], cdt, kind="ExternalInput")
    wash_d = nc.dram_tensor("wash", [H, 32, 32], f8, kind="ExternalInput")
    wgbc_d = nc.dram_tensor("wgbc", [H, P], f8, kind="ExternalInput")
    w1_d = nc.dram_tensor("w1T", [H, 64], f32, kind="ExternalInput")
    w2_d = nc.dram_tensor("w2T", [64, 32], f32, kind="ExternalInput")
    w3_d = nc.dram_tensor("w3T", [32, 1], f32, kind="ExternalInput")
    b1_d = nc.dram_tensor("b1", [64, 1], f32, kind="ExternalInput")
    b2_d = nc.dram_tensor("b2", [32, 1], f32, kind="ExternalInput")
    b3_d = nc.dram_tensor("b3", [1, 1], f32, kind="ExternalInput")

    out_d = nc.dram_tensor("out", [1, BC], f32, kind="ExternalOutput")
    DBG = bool(int(os.environ.get("DIEN_DBG", "0")))
    if DBG:
        dbg_hA_d = nc.dram_tensor("dbg_hA", [H, BC], f32, kind="ExternalOutput")
        dbg_sc_d = nc.dram_tensor("dbg_sc", [P, BC], f32, kind="ExternalOutput")
        dbg_at_d = nc.dram_tensor("dbg_at", [P, BC], f32, kind="ExternalOutput")
        dbg_hB_d = nc.dram_tensor("dbg_hB", [H, BC], f32, kind="ExternalOutput")
        dbg_rz_d = nc.dram_tensor("dbg_rz", [P, BC], f32, kind="ExternalOutput")
        dbg_nb_d = nc.dram_tensor("dbg_nb", [P, BC], f32, kind="ExternalOutput")

    # DRAM scratch for pass B reload
    x16_d = nc.dram_tensor("x16_scratch", [Tsteps, E, BC], cdt, kind="Internal")
    x8_d = nc.dram_tensor("x8_scratch", [Tsteps, E, BC], f8, kind="Internal")

    TT0 = min(P, Tsteps)
    TT1 = Tsteps - TT0
    HS = 16.0      # fp8 activation scale (x and h stored *16)
    RS = 1.0 / HS

    with tile.TileContext(nc) as tc:
        with (
            tc.tile_pool(name="const", bufs=1) as cp,
            tc.tile_pool(name="gat", bufs=3) as gp,
            tc.tile_pool(name="xt", bufs=4) as xp,
            tc.tile_pool(name="xh", bufs=4) as x8p,
            tc.tile_pool(name="hh", bufs=2) as hp,
            tc.tile_pool(name="ew", bufs=2) as ep,
            tc.tile_pool(name="small", bufs=4) as sp,
            tc.tile_pool(name="smax", bufs=1) as mp,
        ):
            # ---------------- constants / weights ----------------
            wst1r = cp.tile([E, 2, H], f8)
            wst1z = cp.tile([E, 2, H], f8)
            wst2r = cp.tile([E, 2, H], f8)
            wst2z = cp.tile([E, 2, H], f8)
            win1 = cp.tile([E, H], cdt)
            whn1 = cp.tile([H, H], cdt)
            win2 = cp.tile([E, H], cdt)
            whn2 = cp.tile([H, H], cdt)
            wash = cp.tile([H, 32, 32], f8)
            wgbc = cp.tile([H, P], f8)
            for sb_t, dr in ((wst1r, wst1r_d), (wst1z, wst1z_d),
                             (wst2r, wst2r_d), (wst2z, wst2z_d),
                             (win1, win1_d), (whn1, whn1_d),
                             (win2, win2_d), (whn2, whn2_d),
                             (wash, wash_d), (wgbc, wgbc_d)):
                nc.sync.dma_start(sb_t[:], dr[:])
            w1 = cp.tile([H, 64], f32)
            w2 = cp.tile([64, 32], f32)
            w3 = cp.tile([32, 1], f32)
            b1 = cp.tile([64, 1], f32)
            b2 = cp.tile([32, 1], f32)
            b3 = cp.tile([1, 1], f32)
            for sb_t, dr in ((w1, w1_d), (w2, w2_d), (w3, w3_d),
                             (b1, b1_d), (b2, b2_d), (b3, b3_d)):
                nc.sync.dma_start(sb_t[:], dr[:])
            from concourse.masks import make_identity
            ident = cp.tile([P, P], cdt)
            make_identity(nc, ident[:])
            ones_row = cp.tile([1, P], f32)
            nc.gpsimd.memset(ones_row[:], 1.0)
            ones_col = cp.tile([P, 1], f32)
            nc.gpsimd.memset(ones_col[:], 1.0)
            ones_col_c = cp.tile([P, 1], cdt)
            nc.gpsimd.memset(ones_col_c[:], 1.0)

            # user history indices: partition = b % 128, free = [t, group(4)]
            uh = cp.tile([P, Tsteps, 4], dt.int32)
            nc.sync.dma_start(uh[:], uh_d[:].rearrange("(c p) t -> p t c", p=P))

            # per-chunk hidden state (fp16 master)
            hA = []
            for c in range(NCHUNK):
                h = hp.tile([H, CW], cdt, tag=f"h{c}")
                nc.gpsimd.memset(h[:], 0.0)
                hA.append(h)

            # score staging tiles (fp32, [t, B]) + g row
            sc_big = [mp.tile([P, BC], f32, tag="scb0", name="scb0"),
                      mp.tile([P, BC], f32, tag="scb1", name="scb1")]
            g_row = cp.tile([1, BC], f32)

            def gather_step(t):
                # NOTE: only [P,1]-offset indirect gathers work on HW
                gat = gp.tile([P, 4, E], cdt, tag="gath", bufs=4)
                for g in range(4):
                    nc.gpsimd.indirect_dma_start(
                        out=gat[:, g, :], out_offset=None, in_=emb16_d[:],
                        in_offset=bass.IndirectOffsetOnAxis(
                            ap=uh[:, t, g:g + 1], axis=0))
                return gat

            # ============ phase 0 + pass A (own PSUM scope) ============
            with (
                tc.tile_pool(name="psrzA0", bufs=2, space="PSUM") as prz0,
                tc.tile_pool(name="psrzA1", bufs=2, space="PSUM") as prz1,
                tc.tile_pool(name="psnbA0", bufs=1, space="PSUM") as pnb0,
                tc.tile_pool(name="psnbA1", bufs=1, space="PSUM") as pnb1,
                tc.tile_pool(name="pssc", bufs=1, space="PSUM") as psc,
            ):
                prz = (prz0, prz1)
                pnb = (pnb0, pnb1)

                xh = {}     # t -> XH fp8 tile [P, 2, BC] (x*16 | h*16)
                xt16 = {}   # t -> fp16 xT tile [E, 4, P]

                def new_xh(t):
                    xh[t] = x8p.tile([P, 2, BC], f8, tag="xh8", name="xh8")
                    return xh[t]

                def xpose_step(t, gat):
                    """DMA-xbar transpose [b,4,E] -> [E,4,b]; cast fp8; store."""
                    xT = xp.tile([E, 4, P], cdt, tag="xT", bufs=4, name="xT")
                    nc.sync.dma_start_transpose(
                        xT[:], gat[:].rearrange("p g e -> p (g e)"))
                    xt16[t] = xT
                    xTf = xT[:].rearrange("e g p -> e (g p)")
                    nc.sync.dma_start(x16_d[t], xTf)
                    t8 = new_xh(t)
                    nc.scalar.mul(t8[:, 0, :], xTf, HS)
                    nc.sync.dma_start(x8_d[t], t8[:, 0, :])

                # phase 0: ad embedding -> g row (overlaps pass A start)
                adidx = cp.tile([P, 4], dt.int32)
                nc.sync.dma_start(adidx[:],
                                  ad_d[:].rearrange("(c p) o -> p (c o)", p=P))
                adg = gp.tile([P, 4, E], cdt, tag="adg")
                for g in range(4):
                    nc.gpsimd.indirect_dma_start(
                        out=adg[:, g, :], out_offset=None, in_=emb16_d[:],
                        in_offset=bass.IndirectOffsetOnAxis(
                            ap=adidx[:, g:g + 1], axis=0))
                adT = xp.tile([E, 4, P], cdt, tag="adT")
                nc.sync.dma_start_transpose(
                    adT[:], adg[:].rearrange("p c e -> p (c e)"))
                g_ps = psc.tile([32, BC], f32, tag="sc")
                nc.tensor.matmul(g_ps[0:1, :], ones_col_c[:],
                                 adT[:].rearrange("e c p -> e (c p)"),
                                 start=True, stop=True)
                # scores come out of PSUM *16 (h8 = 16h); fold 1/16 into g
                nc.scalar.mul(g_row[:], g_ps[0:1, :], RS)

                sc32_box = [None]

                def emit_score(ts):
                    """Score matmul for interest state ts (h after step ts),
                    rhs = fp8 h-half of xh[ts+1] (both chunks at once)."""
                    j = ts % 32
                    if j == 0:
                        sc32_box[0] = psc.tile([32, BC], f32, tag="sc",
                                               name="sc32")
                    nc.tensor.matmul(sc32_box[0][0:32, :], wash[:, j, :],
                                     xh[ts + 1][:, 1, :],
                                     start=(j == 0),
                                     stop=(j == 31 or ts == Tsteps - 1),
                                     skip_group_check=True)

                def emit_drain(ts):
                    k = ts // 32
                    dr = sp.tile([32, BC], f32, tag="drain")
                    nc.vector.tensor_copy(dr[:], sc32_box[0][0:32, :])
                    r0 = (32 * k) % P
                    dst = sc_big[0] if 32 * k < P else sc_big[1]
                    nc.sync.dma_start(dst[r0:r0 + 32, :], dr[:])

                # pipeline prologue
                GDEPTH = 4
                gq = {}
                for tpre in range(min(GDEPTH, Tsteps)):
                    gq[tpre] = gather_step(tpre)
                # init xh[0] h-half to zero BEFORE first xpose creates tiles
                for tpre in range(min(2, Tsteps)):
                    xpose_step(tpre, gq.pop(tpre))
                nc.gpsimd.memset(xh[0][:, 1, :], 0.0)

                def a_half(t, c):
                    """r/z DoubleRow + h_n + sigma + m + i_n for (t, c)."""
                    cs = slice(c * CW, (c + 1) * CW)
                    rz = prz[c].tile([P, 2 * CW], f32, tag=f"rz{c}")
                    rhs = xh[t][:, :, cs]
                    nc.tensor.matmul(rz[:, 0:CW], wst1r[:], rhs,
                                     start=True, stop=True, perf_mode=DR)
                    nc.tensor.matmul(rz[:, CW:2 * CW], wst1z[:], rhs,
                                     start=True, stop=True, perf_mode=DR)
                    nb = pnb[c].tile([P, CW], f32, tag=f"nb{c}", bufs=1)
                    if t > 0:
                        nc.tensor.matmul(nb[:], whn1[:], hA[c][:],
                                         start=True, stop=True)
                    rzs = ep.tile([P, 2 * CW], cdt, tag=f"rzs{c}")
                    nc.scalar.activation(rzs[:], rz[:], AF.Sigmoid, scale=RS)
                    if DBG and t == 1 and c == 0:
                        dbg = sp.tile([P, 2 * CW], f32, tag="dbg", name="dbg")
                        nc.vector.tensor_copy(dbg[:], rzs[:])
                        nc.sync.dma_start(dbg_rz_d[:], dbg[:])
                    if t > 0:
                        # in-place: nb <- r * h_n (has_written stays set)
                        nc.vector.tensor_mul(nb[:], rzs[:, 0:CW], nb[:])
                    # accumulates onto the DVE-written m (stop is sim-only)
                    nc.tensor.matmul(nb[:], win1[:],
                                     xt16[t][:].rearrange("e g p -> e (g p)")[:, cs],
                                     start=(t == 0), stop=True,
                                     skip_group_check=True)
                    return (t, c, nb, rzs)

                def b_half(pend):
                    """tanh + blend + h-cast; consumes a_half's state."""
                    t, c, nb, rzs = pend
                    cs = slice(c * CW, (c + 1) * CW)
                    h = hA[c]
                    n_t = ep.tile([P, CW], cdt, tag=f"n{c}")
                    nc.scalar.activation(n_t[:], nb[:], AF.Tanh)
                    if DBG and t == 1 and c == 0:
                        dbg = sp.tile([P, CW], f32, tag="dbg", name="dbg")
                        nc.vector.tensor_copy(dbg[:], n_t[:])
                        nc.sync.dma_start(dbg_nb_d[:, 0:CW], dbg[:])
                    h_new = hp.tile([H, CW], cdt, tag=f"h{c}")
                    if t > 0:
                        d_t = ep.tile([P, CW], cdt, tag=f"d{c}")
                        nc.vector.tensor_sub(d_t[:], n_t[:], h[:])
                        u_t = ep.tile([P, CW], cdt, tag=f"u{c}")
                        nc.vector.tensor_mul(u_t[:], rzs[:, CW:2 * CW], d_t[:])
                        nc.vector.tensor_add(h_new[:], h[:], u_t[:])
                    else:
                        nc.vector.tensor_mul(h_new[:], rzs[:, CW:2 * CW],
                                             n_t[:])
                    hA[c] = h_new
                    if t + 1 not in xh:
                        new_xh(t + 1)
                    nc.vector.tensor_scalar_mul(xh[t + 1][:, 1, cs],
                                                h_new[:], HS)

                pend1 = None
                for t in range(Tsteps):
                    a0 = a_half(t, 0)
                    if pend1 is not None:
                        b_half(pend1)
                    b_half(a0)
                    pend1 = a_half(t, 1)
                    # prefetch: gather t+GDEPTH, transpose+cast t+2
                    if t + GDEPTH < Tsteps:
                        gq[t + GDEPTH] = gather_step(t + GDEPTH)
                    if t + 2 < Tsteps:
                        xpose_step(t + 2, gq.pop(t + 2))
                    if t > 0:
                        emit_score(t - 1)
                        if (t - 1) % 32 == 31:
                            emit_drain(t - 1)
                    xh.pop(t - 1, None)
                    xt16.pop(t - 1, None)
                b_half(pend1)   # c1 final step
                emit_score(Tsteps - 1)
                emit_drain(Tsteps - 1)
                xh.clear()
                xt16.clear()
                if DBG:
                    for c in range(NCHUNK):
                        dbg = sp.tile([H, CW], f32, tag="dbg", name="dbg")
                        nc.vector.tensor_copy(dbg[:], hA[c][:])
                        nc.sync.dma_start(
                            dbg_hA_d[:, c * CW:(c + 1) * CW], dbg[:])
                    nc.sync.dma_start(dbg_sc_d[0:TT0, :], sc_big[0][0:TT0, :])

            # ============ softmax over t, scaled by g (own scope) ============
            at_tiles = [mp.tile([P, BC], cdt, tag="at0", name="at0"),
                        mp.tile([P, BC], cdt, tag="at1", name="at1")]
            with (
                tc.tile_pool(name="psden", bufs=1, space="PSUM") as pden,
                tc.tile_pool(name="psbb", bufs=1, space="PSUM") as pbb,
            ):
                gb_ps = pbb.tile([P, BC], f32, tag="bb")
                nc.tensor.matmul(gb_ps[:], ones_row[:], g_row[:],
                                 start=True, stop=True)
                gb = mp.tile([P, BC], f32, tag="gb")
                nc.vector.tensor_copy(gb[:], gb_ps[:])
                den_ps = pden.tile([1, BC], f32, tag="den")
                ex_tiles = []
                for i, (t0, tl) in enumerate(((0, TT0), (TT0, TT1))):
                    if tl == 0:
                        continue
                    sg = mp.tile([P, BC], f32, tag=f"sg{i}")
                    nc.vector.tensor_mul(sg[:tl, :], sc_big[i][:tl, :],
                                         gb[:tl, :])
                    exp_t = mp.tile([P, BC], f32, tag=f"ex{i}")
                    nc.scalar.activation(exp_t[:tl, :], sg[:tl, :], AF.Exp)
                    nc.tensor.matmul(den_ps[:], ones_col[:tl, :], exp_t[:tl, :],
                                     start=(i == 0), stop=(tl + t0 == Tsteps))
                    ex_tiles.append((exp_t, t0, tl))
                den = sp.tile([1, BC], f32, tag="den")
                nc.vector.tensor_copy(den[:], den_ps[:])
                rden = sp.tile([1, BC], f32, tag="rden")
                nc.vector.reciprocal(rden[:], den[:])
                rb_ps = pbb.tile([P, BC], f32, tag="bb")
                nc.tensor.matmul(rb_ps[:], ones_row[:], rden[:],
                                 start=True, stop=True)
                rb = mp.tile([P, BC], f32, tag="rb")
                nc.vector.tensor_copy(rb[:], rb_ps[:])
                for exp_t, t0, tl in ex_tiles:
                    dst = at_tiles[0] if t0 == 0 else at_tiles[1]
                    nc.vector.tensor_mul(dst[:tl, :], exp_t[:tl, :], rb[:tl, :])
                if DBG:
                    dbg = mp.tile([P, BC], f32, tag="dbgat", name="dbgat")
                    nc.vector.tensor_copy(dbg[0:TT0, :], at_tiles[0][0:TT0, :])
                    nc.sync.dma_start(dbg_at_d[0:TT0, :], dbg[0:TT0, :])

            # ============ pass B: AUGRU (own scope) ============
            with (
                tc.tile_pool(name="psrzB0", bufs=1, space="PSUM") as qrz0,
                tc.tile_pool(name="psrzB1", bufs=1, space="PSUM") as qrz1,
                tc.tile_pool(name="psnbB0", bufs=1, space="PSUM") as qnb0,
                tc.tile_pool(name="psnbB1", bufs=1, space="PSUM") as qnb1,
            ):
                qrz = (qrz0, qrz1)
                qnb = (qnb0, qnb1)

                xhb = {}    # t -> fp8 [P, 2, BC] (x*16 | h*16)
                xb16 = {}   # t -> fp16 [E, BC]

                def new_xhb(t):
                    xhb[t] = x8p.tile([P, 2, BC], f8, tag="xhb8", bufs=5,
                                      name="xhb8")
                    return xhb[t]

                def b_inputs(t):
                    """Prefetchable pass-B inputs for step t."""
                    t8 = xhb.get(t) or new_xhb(t)
                    nc.sync.dma_start(t8[:, 0, :], x8_d[t])
                    xT = xp.tile([E, BC], cdt, tag="bxT", bufs=5, name="bxT")
                    nc.sync.dma_start(xT[:], x16_d[t])
                    xb16[t] = xT
                    abt = []
                    tt = 0 if t < P else 1
                    for c in range(NCHUNK):
                        arow = sp.tile([1, CW], cdt, tag=f"ar{c}", bufs=5)
                        nc.sync.dma_start(arow[:],
                                          at_tiles[tt][t % P:t % P + 1,
                                                       c * CW:(c + 1) * CW])
                        abt.append(arow)
                    return abt

                # prologue: cast pass-A final h into xhb[0] h-half
                new_xhb(0)
                for c in range(NCHUNK):
                    nc.scalar.mul(xhb[0][:, 1, c * CW:(c + 1) * CW],
                                  hA[c][:], HS)
                BDEPTH = 3
                inq = {}
                for tpre in range(min(BDEPTH, Tsteps)):
                    inq[tpre] = b_inputs(tpre)

                def b_a_half(t, c):
                    """rg + r/z DoubleRow + h_n + sigma + m + i_n + gamma."""
                    cs = slice(c * CW, (c + 1) * CW)
                    arow = inq[t][c]
                    h = hA[c]
                    # bank0: [r | z] (one group); bank1: [rg | pad]
                    rzg = qrz[c].tile([P, 4 * CW], f32, tag=f"brz{c}")
                    rhs = xhb[t][:, :, cs]
                    nc.tensor.matmul(rzg[:, 0:CW], wst2r[:], rhs,
                                     start=True, stop=True, perf_mode=DR)
                    nc.tensor.matmul(rzg[:, CW:2 * CW], wst2z[:], rhs,
                                     start=True, stop=True, perf_mode=DR)
                    nc.tensor.matmul(rzg[:, 2 * CW:3 * CW], wgbc[:],
                                     xhb[t][:, 1, cs], start=True, stop=True)
                    nb = qnb[c].tile([P, CW], f32, tag=f"nb{c}", bufs=1)
                    nc.tensor.matmul(nb[:], whn2[:], h[:],
                                     start=True, stop=True)
                    rzs = ep.tile([P, 3 * CW], cdt, tag=f"brzs{c}")
                    nc.scalar.activation(rzs[:], rzg[:, 0:3 * CW], AF.Sigmoid,
                                         scale=RS)
                    grow = sp.tile([1, CW], cdt, tag=f"grow{c}")
                    nc.vector.tensor_mul(grow[:], rzs[0:1, 2 * CW:3 * CW],
                                         arow[:])
                    gb = ep.tile([P, CW], cdt, tag=f"gb{c}")
                    nc.gpsimd.partition_broadcast(gb[:], grow[:])
                    nc.vector.tensor_mul(nb[:], rzs[:, 0:CW], nb[:])
                    nc.tensor.matmul(nb[:], win2[:], xb16[t][:, cs],
                                     start=False, stop=True,
                                     skip_group_check=True)
                    return (t, c, nb, rzs, gb)

                def b_b_half(pend):
                    t, c, nb, rzs, gb = pend
                    cs = slice(c * CW, (c + 1) * CW)
                    h = hA[c]
                    n_t = ep.tile([P, CW], cdt, tag=f"n{c}")
                    nc.scalar.activation(n_t[:], nb[:], AF.Tanh)
                    d_t = ep.tile([P, CW], cdt, tag=f"d{c}")
                    nc.vector.tensor_sub(d_t[:], n_t[:], h[:])
                    u_t = ep.tile([P, CW], cdt, tag=f"u{c}")
                    nc.vector.tensor_mul(u_t[:], rzs[:, CW:2 * CW], d_t[:])
                    e_t = ep.tile([P, CW], cdt, tag=f"e{c}")
                    nc.vector.tensor_mul(e_t[:], gb[:], u_t[:])
                    h_new = hp.tile([H, CW], cdt, tag=f"h{c}")
                    nc.vector.tensor_add(h_new[:], h[:], e_t[:])
                    hA[c] = h_new
                    if t + 1 < Tsteps:
                        if t + 1 not in xhb:
                            new_xhb(t + 1)
                        nc.scalar.mul(xhb[t + 1][:, 1, cs], h_new[:], HS)

                pend1 = None
                for t in range(Tsteps):
                    a0 = b_a_half(t, 0)
                    if pend1 is not None:
                        b_b_half(pend1)
                    b_b_half(a0)
                    pend1 = b_a_half(t, 1)
                    if t + BDEPTH < Tsteps:
                        inq[t + BDEPTH] = b_inputs(t + BDEPTH)
                    inq.pop(t - 1, None)
                    xhb.pop(t - 1, None)
                    xb16.pop(t - 1, None)
                b_b_half(pend1)
                inq.clear()
                xhb.clear()
                xb16.clear()
                if DBG:
                    for c in range(NCHUNK):
                        dbg = sp.tile([H, CW], f32, tag="dbg", name="dbg")
                        nc.vector.tensor_copy(dbg[:], hA[c][:])
                        nc.sync.dma_start(
                            dbg_hB_d[:, c * CW:(c + 1) * CW], dbg[:])

            # ============ MLP head (own scope) ============
            with tc.tile_pool(name="psmlp", bufs=1, space="PSUM") as pm:
                x1_ps = pm.tile([64, BC], f32, tag="x1")
                for c in range(NCHUNK):
                    hf = ep.tile([H, CW], f32, tag=f"hf{c}")
                    nc.vector.tensor_copy(hf[:], hA[c][:])
                    nc.tensor.matmul(x1_ps[:, c * CW:(c + 1) * CW], w1[:], hf[:],
                                     start=True, stop=(c == NCHUNK - 1),
                                     skip_group_check=True)
                x1 = ep.tile([64, BC], f32, tag="mlp1")
                nc.scalar.activation(x1[:], x1_ps[:], AF.Relu, bias=b1[:, 0:1])
                x2_ps = pm.tile([32, BC], f32, tag="x2")
                nc.tensor.matmul(x2_ps[:], w2[:], x1[:], start=True, stop=True)
                x2 = ep.tile([32, BC], f32, tag="mlp2")
                nc.scalar.activation(x2[:], x2_ps[:], AF.Relu, bias=b2[:, 0:1])
                y_ps = pm.tile([1, BC], f32, tag="y")
                nc.tensor.matmul(y_ps[:], w3[:], x2[:], start=True, stop=True)
                y = sp.tile([1, BC], f32, tag="y")
                nc.scalar.activation(y[:], y_ps[:], AF.Identity, bias=b3[:, 0:1])
                nc.sync.dma_start(out_d[:], y[:])

    return nc


def _prep_inputs(user_hist, ad_feature, emb, Wih1, Whh1, bih1, bhh1, wa, ba,
                 Wih2, Whh2, bih2, bhh2, wg, bg, W1, b1, W2, b2, W3, b3,
                 Tsteps):
    import ml_dtypes
    np8 = ml_dtypes.float8_e4m3
    f32 = np.float32
    f16 = np.float16
    assert not (np.any(bih1) or np.any(bhh1) or np.any(bih2) or np.any(bhh2)), \
        "nonzero GRU biases not supported by this kernel build"
    assert float(np.asarray(ba)) == 0.0, "nonzero attention bias not supported"
    assert float(np.asarray(bg)) == 0.0, "nonzero AUGRU gate bias not supported"

    def stack8(W, gate, neg=False):
        # W: [3H, X] torch (r,z,n); gate 0=r 1=z -> lhsT [X, 2, H] fp8
        Wi = np.asarray(W, f32)[gate * H:(gate + 1) * H, :]   # [H, E|H]
        if neg:
            Wi = -Wi
        return np.ascontiguousarray(Wi.T).astype(np8)

    def mkstack(Wih, Whh, gate, neg):
        a = stack8(Wih, gate, neg)    # [E, H]
        b = stack8(Whh, gate, neg)    # [H, H]
        return np.ascontiguousarray(np.stack([a, b], axis=1))  # [E, 2, H]

    def ngate(W):
        Wn = np.asarray(W, f32)[2 * H:3 * H, :]
        return np.ascontiguousarray(Wn.T).astype(f16)

    wash = np.zeros((H, 32, 32), f32)
    for j in range(32):
        wash[:, j, j] = np.asarray(wa, f32)

    common = {
        "emb16": np.ascontiguousarray(emb).astype(f16),
        "wst1r": mkstack(Wih1, Whh1, 0, False),
        "wst1z": mkstack(Wih1, Whh1, 1, True),
        "wst2r": mkstack(Wih2, Whh2, 0, False),
        "wst2z": mkstack(Wih2, Whh2, 1, True),
        "win1": ngate(Wih1), "whn1": ngate(Whh1),
        "win2": ngate(Wih2), "whn2": ngate(Whh2),
        "wash": wash.astype(np8),
        "wgbc": np.ascontiguousarray(
            np.tile(np.asarray(wg, f32).reshape(H, 1), (1, P))).astype(np8),
        "w1T": np.ascontiguousarray(W1.T, dtype=f32),
        "w2T": np.ascontiguousarray(W2.T, dtype=f32),
        "w3T": np.ascontiguousarray(W3.T, dtype=f32),
        "b1": np.ascontiguousarray(b1.reshape(64, 1), dtype=f32),
        "b2": np.ascontiguousarray(b2.reshape(32, 1), dtype=f32),
        "b3": np.ascontiguousarray(b3.reshape(1, 1), dtype=f32),
    }

    in_maps = []
    for c in range(NCORES):
        rows = slice(c * BC, (c + 1) * BC)
        m = dict(common)
        m["user_hist"] = np.ascontiguousarray(user_hist[rows, :Tsteps], dtype=np.int32)
        m["ad_feature"] = np.ascontiguousarray(
            ad_feature[rows].reshape(BC, 1), dtype=np.int32)
        in_maps.append(m)
    return in_maps


_CACHE = {}


def kernel(user_hist, ad_feature, emb, Wih1, Whh1, bih1, bhh1, wa, ba,
           Wih2, Whh2, bih2, bhh2, wg, bg, W1, b1, W2, b2, W3, b3,
           _trace=False, _tsteps=None):
    import concourse.bacc as bacc
    from concourse.bass_utils import run_bass_kernel_spmd

    Tsteps = _tsteps or T
    key = Tsteps
    if key not in _CACHE:
        nc = bacc.Bacc("TRN2", num_devices=1, enable_asserts=True)
        _build(nc, Tsteps)
        nc.compile()
        _CACHE[key] = nc
    nc = _CACHE[key]

    in_maps = _prep_inputs(user_hist, ad_feature, emb, Wih1, Whh1, bih1, bhh1,
                           wa, ba, Wih2, Whh2, bih2, bhh2, wg, bg,
                           W1, b1, W2, b2, W3, b3, Tsteps)
    r = run_bass_kernel_spmd(nc, in_maps, core_ids=list(range(NCORES)),
                             trace=_trace)
    out = np.concatenate(
        [np.asarray(r.results[c]["out"]).reshape(BC, 1) for c in range(NCORES)],
        axis=0)
    if _trace:
        kernel._last_result = r
    return out.astype(np.float32)
